# revision 17
# baseline (speedup 1.0000x reference)
"""Single-head attention (B=4, S=4096, E=2048, d=128) on 8 trn2 cores.

Sharding: core c handles (batch b = c//2, seq half h = c%2). Each core
projects q/k/v for its own 2048-row half; the pair (2b, 2b+1) exchanges
K then V via two small 2-core AllGathers (the collective DMA path runs
at ~32 GB/s, so latency scales with payload: two 0.5MB gathers, K first
since exp-B is gated on peer K). Softmax over keys is permutation-
invariant, so per-core key order (own-first) is harmless.

Bias algebra: k-bias shifts every key score of a query by a per-query
constant -> softmax-invariant -> dropped. v-bias adds bv to the output
post-normalization -> added on the host. Only the q-bias is applied on
device (folded into the q PSUM evacuation on the DVE).

Hard constraints that shape the schedule: the x load is DMA-bound
(8.4MB at ~330GB/s aggregate = ~29us, all of x needed by every
projection), the Tensor queue is strict FIFO (emission order = run
order), and ACT's 73us exp stream is gated on q (pass A) / peer K
(pass B). So:
  warmup MMs | x on 3 queues, w interleaved by first use
  load phase: k-FULL + v/q row-half0 matmuls chase the arriving tiles
    (k completes the moment x lands ~40us -> CC-K doorbell ~44us)
  scores+exp A qb0/qb1 | q half1 | scores A qb2/qb3 | v half1 | CC-V
  own v transposes (PE) | peer v transposed from DRAM by the DMA xbar
  pv_tree(A qb) and scores+exp(B qb) interleaved | pv_tree B qb0..3
Denominators: DVE halving tree over the contiguous exp region
[128, 8192], then a ones-column matmul into partition 32*qb of a
rotating PSUM tile (tile_position); pass A/B summed on the DVE in SBUF.
"""

import numpy as np
import ml_dtypes

import concourse.tile as tile
from concourse import bacc, mybir
from concourse.bass_utils import run_bass_kernel_spmd

N_CORES = 8
B, S, E, D = 4, 4096, 2048, 128
HALF = S // 2  # queries / own keys per core
QB = 512  # query block (PSUM bank width in fp32)
NE = E // 128  # 16 e-chunks
NQB = HALF // QB  # 4 query blocks
SCALE = 1.0 / float(np.sqrt(D))

BF16 = mybir.dt.bfloat16
F32 = mybir.dt.float32
AF = mybir.ActivationFunctionType
GROUPS = [[2 * i, 2 * i + 1] for i in range(N_CORES // 2)]

_CACHE = {}


def _build():
    nc = bacc.Bacc(
        trn_type="TRN2", target_bir_lowering=False, debug=False, num_devices=N_CORES
    )

    x_d = nc.dram_tensor("xt", [E, HALF], BF16, kind="ExternalInput").ap()
    # w packed cg-major: [128, cg(3) * e(16) * 128], cg order (k, q, v)
    w_d = nc.dram_tensor("w", [128, 3 * NE * 128], BF16, kind="ExternalInput").ap()
    bias_d = nc.dram_tensor("bias_q", [D, 1], F32, kind="ExternalInput").ap()
    peer_d = nc.dram_tensor("peer", [1, 1], mybir.dt.uint32, kind="ExternalInput").ap()
    out_d = nc.dram_tensor("out_t", [D, HALF], F32, kind="ExternalOutput").ap()
    sums_d = nc.dram_tensor("sums", [1, HALF], F32, kind="ExternalOutput").ap()

    with tile.TileContext(nc) as tc:
        with (
            tc.tile_pool(name="xt", bufs=16) as xt_pool,
            tc.tile_pool(name="wsb", bufs=1) as w_pool,
            tc.tile_pool(name="persist", bufs=1) as persist,
            tc.tile_pool(name="exp", bufs=4) as exp_pool,
            tc.tile_pool(name="comb", bufs=2) as comb_pool,
            tc.tile_pool(name="dram", bufs=1, space="DRAM") as dram_pool,
            tc.tile_pool(name="ps", bufs=4, space="PSUM") as ps_pool,
        ):
            # ---- warmup fodder first so the PE can start immediately ----
            junk = persist.tile([128, QB], BF16, tag="junk")
            nc.gpsimd.memset(junk[:], 0.0)
            ps_warm = ps_pool.tile([128, 2 * QB], F32, tag="ps")
            for _ in range(6):
                nc.tensor.matmul(
                    ps_warm[:, 0:QB], lhsT=junk[:, 0:128], rhs=junk[:],
                    start=True, stop=True,
                )
            ones_col = persist.tile([128, 1], BF16, tag="ones")
            nc.gpsimd.memset(ones_col[:], 1.0)
            bias_sb = persist.tile([D, 1], F32, tag="bias")
            nc.gpsimd.dma_start(bias_sb[:], bias_d[:])

            # peer slot register (host supplies 1 on even cores, 0 on odd)
            peer_reg = nc.sync.alloc_register("peer_slot")
            nc.sync.reg_load(peer_reg, peer_d[0:1, 0:1])
            peer_val = nc.sync.snap(peer_reg, donate=True, min_val=0, max_val=1)

            # ---- weight / x loads; w interleaved by first use ----
            w_sb = w_pool.tile([128, 3 * NE * 128], BF16, tag="w")
            WG = NE * 128  # one cg = 2048 cols

            def w_ap(cg, e):
                return w_sb[:, cg * WG + e * 128 : cg * WG + (e + 1) * 128]

            def w_load(cg, half, eng):
                lo = cg * WG + half * WG // 2
                eng.dma_start(w_sb[:, lo : lo + WG // 2], w_d[:, lo : lo + WG // 2])

            w_load(0, 0, nc.sync)
            w_load(0, 1, nc.scalar)
            xt = {}
            x_engs = (nc.sync, nc.scalar, nc.gpsimd)
            for e in range(NE):
                t = xt_pool.tile([128, HALF], BF16, tag="xt")
                x_engs[e % 3].dma_start(t[:], x_d[e * 128 : (e + 1) * 128, :])
                xt[e] = t
                if e == 0:
                    w_load(2, 0, nc.sync)
                    w_load(2, 1, nc.scalar)
                if e == 5:
                    w_load(1, 0, nc.sync)
                    w_load(1, 1, nc.scalar)

            # ---- persistent activations ----
            qT = persist.tile([D, HALF], BF16, tag="qT")
            k_sb = persist.tile([D, S], BF16, tag="k_sb")  # [own kT | peer kT]
            vT_own = persist.tile([D, HALF], BF16, tag="vT_own")
            v_sb = persist.tile([128, (S // 128) * D], BF16, tag="v_sb")
            sums_stage = persist.tile([128, QB], F32, tag="sums_stage")
            o_stage = persist.tile([D, HALF], F32, tag="o_stage")

            def k_ap(c):  # kT chunk c (d on partitions); own 0-15, peer 16-31
                return k_sb[:, c * 128 : (c + 1) * 128]

            # ---- load phase: k-FULL + v-FULL chase the arriving x tiles
            # (both gate the combined K|V AllGather; v lags 2 e-chunks so
            # the wv DMA arrives) ----
            ps_ka = ps_pool.tile([128, 2 * QB], F32, tag="ps")
            ps_kb = ps_pool.tile([128, 2 * QB], F32, tag="ps")
            ps_va = ps_pool.tile([128, 2 * QB], F32, tag="ps")
            ps_vb = ps_pool.tile([128, 2 * QB], F32, tag="ps")
            pk = [ps_ka, ps_kb]
            pv = [ps_va, ps_vb]
            LAG = 2
            for step in range(NE + LAG):
                for cg, pd, e in ((0, pk, step), (2, pv, step - LAG)):
                    if 0 <= e < NE:
                        for blk in range(4):
                            nc.tensor.matmul(
                                pd[blk // 2][:, (blk % 2) * QB : (blk % 2 + 1) * QB],
                                lhsT=w_ap(cg, e),
                                rhs=xt[e][:, blk * QB : (blk + 1) * QB],
                                start=(e == 0),
                                stop=(e == NE - 1),
                            )
            for i in range(2):
                nc.vector.tensor_copy(k_sb[:, i * 2 * QB : (i + 1) * 2 * QB], pk[i][:])
            for i in range(2):
                nc.vector.tensor_copy(
                    vT_own[:, i * 2 * QB : (i + 1) * 2 * QB], pv[i][:]
                )

            # ---- combined K|V exchange (1MB AllGather, one doorbell) ----
            cc_in = dram_pool.tile([D, S], BF16, tag="cc_in")
            cc_out = dram_pool.tile([2, D, S], BF16, tag="cc_out")
            nc.sync.dma_start(cc_in[:, HALF:S], vT_own[:])
            nc.gpsimd.dma_start(cc_in[:, 0:HALF], k_sb[:, 0:HALF])
            # own v transposed from the DRAM staging copy by the DMA xbar
            for c in range(16):
                nc.sync.dma_start_transpose(
                    v_sb[:, c * D : (c + 1) * D],
                    cc_in[:, HALF + c * 128 : HALF + (c + 1) * 128],
                )
            nc.gpsimd.collective_compute(
                "AllGather",
                mybir.AluOpType.bypass,
                replica_groups=GROUPS,
                ins=[cc_in.opt()],
                outs=[cc_out.opt()],
            )
            nc.sync.dma_start(k_sb[:, HALF:S], cc_out[peer_val][:, 0:HALF])
            for c in range(16):
                nc.sync.dma_start_transpose(
                    v_sb[:, (16 + c) * D : (17 + c) * D],
                    cc_out[peer_val][:, HALF + c * 128 : HALF + (c + 1) * 128],
                )

            # ---- attention machinery ----
            exp_regions = {}

            def scores_exp(qb, p):
                """Scores + exp for all 8 k-pairs of pass p, query block qb."""
                ex = exp_pool.tile([128, 16 * QB], BF16, tag="exp")
                exp_regions[(qb, p)] = ex
                q_ap = qT[:, qb * QB : (qb + 1) * QB]
                for kp in range(8):
                    ps = ps_pool.tile([128, 2 * QB], F32, tag="ps")
                    for half in range(2):
                        nc.tensor.matmul(
                            ps[:, half * QB : (half + 1) * QB],
                            lhsT=k_ap(16 * p + 2 * kp + half),
                            rhs=q_ap,
                            start=True,
                            stop=True,
                        )
                    nc.scalar.activation(
                        ex[:, kp * 2 * QB : (kp + 1) * 2 * QB], ps[:], AF.Exp,
                        scale=SCALE,
                    )

            def project_half(half, evac):
                """One 1024-row half of the q projection."""
                ps = ps_pool.tile([128, 2 * QB], F32, tag="ps")
                for e in range(NE):
                    for blk in range(2):
                        nc.tensor.matmul(
                            ps[:, blk * QB : (blk + 1) * QB],
                            lhsT=w_ap(1, e),
                            rhs=xt[e][:, (2 * half + blk) * QB : (2 * half + blk + 1) * QB],
                            start=(e == 0),
                            stop=(e == NE - 1),
                        )
                evac(ps[:])

            def pv_tree(qb, p):
                """PV accumulation + denominator tree for pass p, block qb."""
                ex = exp_regions.pop((qb, p))
                ps_o = ps_pool.tile([128, 2 * QB], F32, tag="ps")
                for kp in range(8):
                    for half in range(2):
                        c = 16 * p + 2 * kp + half
                        off = kp * 2 * QB + half * QB
                        nc.tensor.matmul(
                            ps_o[:, 0:QB],
                            lhsT=v_sb[:, c * D : (c + 1) * D],
                            rhs=ex[:, off : off + QB],
                            start=(kp == 0 and half == 0),
                            stop=(kp == 7 and half == 1),
                        )
                # output evacuation first: it is the pass-B critical tail
                o_sl = o_stage[:, qb * QB : (qb + 1) * QB]
                if p == 0:
                    nc.vector.tensor_copy(o_sl, ps_o[:, 0:QB])
                else:
                    nc.vector.tensor_add(o_sl, o_sl, ps_o[:, 0:QB])
                    nc.sync.dma_start(out_d[:, qb * QB : (qb + 1) * QB], o_sl)
                # halving tree; LA depends on kp0-3's exp, LB on kp4-7's, so
                # only M/N/C3 (~2.2us DVE) trail the final exp of the pass.
                W4, W2, W1 = 4 * QB, 2 * QB, QB
                cb = comb_pool.tile([128, 15 * QB], BF16, tag="comb")
                la, lb, m, n, c3 = (
                    cb[:, 0:W4],
                    cb[:, W4 : 2 * W4],
                    cb[:, 2 * W4 : 3 * W4],
                    cb[:, 3 * W4 : 3 * W4 + W2],
                    cb[:, 3 * W4 + W2 : 3 * W4 + W2 + W1],
                )
                nc.vector.tensor_add(la, ex[:, 0:W4], ex[:, W4 : 2 * W4])
                nc.vector.tensor_add(lb, ex[:, 2 * W4 : 3 * W4], ex[:, 3 * W4 : 4 * W4])
                nc.vector.tensor_add(m, la, lb)
                nc.vector.tensor_add(n, m[:, 0:W2], m[:, W2 : 2 * W2])
                nc.vector.tensor_add(c3, n[:, 0:W1], n[:, W1 : 2 * W1])
                ps_s = ps_pool.tile([128, 2 * QB], F32, tag="ps")
                nc.tensor.matmul(
                    ps_s[32 * qb : 32 * qb + 1, 0:QB],
                    lhsT=ones_col[:],
                    rhs=c3,
                    start=True,
                    stop=True,
                    tile_position=(0, 32 * qb),
                )
                s_sl = sums_stage[32 * qb : 32 * qb + 1, :]
                if p == 0:
                    nc.vector.tensor_copy(s_sl, ps_s[32 * qb : 32 * qb + 1, 0:QB])
                else:
                    nc.vector.tensor_add(s_sl, s_sl, ps_s[32 * qb : 32 * qb + 1, 0:QB])
                    nc.sync.dma_start(sums_d[0:1, qb * QB : (qb + 1) * QB], s_sl)

            # ---- pass A emission: q halves, exp stream chasing each ----
            project_half(
                0,
                lambda ps: nc.vector.tensor_scalar_add(
                    qT[:, 0 : 2 * QB], ps, bias_sb[:]
                ),
            )
            scores_exp(0, 0)
            scores_exp(1, 0)
            project_half(
                1,
                lambda ps: nc.vector.tensor_scalar_add(
                    qT[:, 2 * QB : 4 * QB], ps, bias_sb[:]
                ),
            )
            scores_exp(2, 0)
            scores_exp(3, 0)

            # ---- interleave pass-A PV with pass-B scores/exp ----
            for qb in range(NQB):
                pv_tree(qb, 0)
                scores_exp(qb, 1)
            for qb in range(NQB):
                pv_tree(qb, 1)

    nc.compile()
    return nc


def _prep_inputs(x, W, b):
    """Host-side sharding prep: cast bf16, transpose to xT, pack w cg-major."""
    b_f = np.asarray(b, dtype=np.float32)
    bias_q = np.ascontiguousarray(b_f[0:D].reshape(D, 1))  # q bias column
    # W [E, 3D] -> [128p, cg(3), e(16), 128] with cg order (k, q, v)
    w4 = np.asarray(W).astype(ml_dtypes.bfloat16).reshape(NE, 128, 3, D)
    w_bf = np.ascontiguousarray(
        w4.transpose(1, 2, 0, 3)[:, [1, 0, 2], :, :].reshape(128, 3 * NE * D)
    )
    in_maps = []
    for bb in range(B):
        xt_full = np.ascontiguousarray(
            np.asarray(x[bb]).astype(ml_dtypes.bfloat16).T
        )  # [E, S]
        for h in range(2):
            xc = np.ascontiguousarray(xt_full[:, h * HALF : (h + 1) * HALF])
            peer = np.array([[1 - h]], dtype=np.uint32)
            in_maps.append(
                {"xt": xc, "w": w_bf, "bias_q": bias_q, "peer": peer}
            )
    return in_maps


def _run(in_maps, trace=False, trace_kwargs=None):
    if "nc" not in _CACHE:
        _CACHE["nc"] = _build()
    return run_bass_kernel_spmd(
        _CACHE["nc"],
        in_maps,
        list(range(N_CORES)),
        trace=trace,
        **(trace_kwargs or {}),
    )


def kernel(x, W, b):
    in_maps = _prep_inputs(x, W, b)
    res = None
    for attempt in range(3):
        try:
            res = _run(in_maps)
            break
        except Exception:
            if attempt == 2:
                raise
    bv = np.asarray(b, dtype=np.float32)[2 * D : 3 * D]  # v bias, host-applied
    out = np.empty((B, S, D), dtype=np.float32)
    for c in range(N_CORES):
        bb, h = c // 2, c % 2
        o_t = res.results[c]["out_t"]  # [D, HALF]
        sums = res.results[c]["sums"]  # [1, HALF]
        out[bb, h * HALF : (h + 1) * HALF, :] = (o_t / sums).T + bv
    return out


# revision 18
# speedup vs baseline: 1.1719x; 1.1719x over previous
"""Single-head attention (B=4, S=4096, E=2048, d=128) on 8 trn2 cores.

Sharding: core c handles (batch b = c//2, seq half h = c%2). Each core
projects q/k/v for its own 2048-row half; the pair (2b, 2b+1) exchanges
K then V via two small 2-core AllGathers (the collective DMA path runs
at ~32 GB/s, so latency scales with payload: two 0.5MB gathers, K first
since exp-B is gated on peer K). Softmax over keys is permutation-
invariant, so per-core key order (own-first) is harmless.

Bias algebra: k-bias shifts every key score of a query by a per-query
constant -> softmax-invariant -> dropped. v-bias adds bv to the output
post-normalization -> added on the host. Only the q-bias is applied on
device (folded into the q PSUM evacuation on the DVE).

Hard constraints that shape the schedule: the x load is DMA-bound
(8.4MB at ~330GB/s aggregate = ~29us, all of x needed by every
projection), the Tensor queue is strict FIFO (emission order = run
order), and ACT's 73us exp stream is gated on q (pass A) / peer K
(pass B). So:
  warmup MMs | x on 3 queues, w interleaved by first use
  load phase: k-FULL + v/q row-half0 matmuls chase the arriving tiles
    (k completes the moment x lands ~40us -> CC-K doorbell ~44us)
  scores+exp A qb0/qb1 | q half1 | scores A qb2/qb3 | v half1 | CC-V
  own v transposes (PE) | peer v transposed from DRAM by the DMA xbar
  pv_tree(A qb) and scores+exp(B qb) interleaved | pv_tree B qb0..3
Denominators: DVE halving tree over the contiguous exp region
[128, 8192], then a ones-column matmul into partition 32*qb of a
rotating PSUM tile (tile_position); pass A/B summed on the DVE in SBUF.
"""

import numpy as np
import ml_dtypes

import concourse.tile as tile
from concourse.masks import make_identity
from concourse import bacc, mybir
from concourse.bass_utils import run_bass_kernel_spmd

N_CORES = 8
B, S, E, D = 4, 4096, 2048, 128
HALF = S // 2  # queries / own keys per core
QB = 512  # query block (PSUM bank width in fp32)
NE = E // 128  # 16 e-chunks
NQB = HALF // QB  # 4 query blocks
SCALE = 1.0 / float(np.sqrt(D))

BF16 = mybir.dt.bfloat16
F32 = mybir.dt.float32
AF = mybir.ActivationFunctionType
GROUPS = [[2 * i, 2 * i + 1] for i in range(N_CORES // 2)]

_CACHE = {}


def _build():
    nc = bacc.Bacc(
        trn_type="TRN2", target_bir_lowering=False, debug=False, num_devices=N_CORES
    )

    x_d = nc.dram_tensor("xt", [E, HALF], BF16, kind="ExternalInput").ap()
    # w packed cg-major: [128, cg(3) * e(16) * 128], cg order (k, q, v)
    w_d = nc.dram_tensor("w", [128, 3 * NE * 128], BF16, kind="ExternalInput").ap()
    bias_d = nc.dram_tensor("bias_q", [D, 1], F32, kind="ExternalInput").ap()
    peer_d = nc.dram_tensor("peer", [1, 1], mybir.dt.uint32, kind="ExternalInput").ap()
    out_d = nc.dram_tensor("out_t", [D, HALF], F32, kind="ExternalOutput").ap()
    sums_d = nc.dram_tensor("sums", [1, HALF], F32, kind="ExternalOutput").ap()

    with tile.TileContext(nc) as tc:
        with (
            tc.tile_pool(name="xt", bufs=16) as xt_pool,
            tc.tile_pool(name="wsb", bufs=1) as w_pool,
            tc.tile_pool(name="persist", bufs=1) as persist,
            tc.tile_pool(name="exp", bufs=4) as exp_pool,
            tc.tile_pool(name="comb", bufs=2) as comb_pool,
            tc.tile_pool(name="dram", bufs=1, space="DRAM") as dram_pool,
            tc.tile_pool(name="ps", bufs=4, space="PSUM") as ps_pool,
        ):
            # ---- warmup fodder first so the PE can start immediately ----
            junk = persist.tile([128, QB], BF16, tag="junk")
            nc.gpsimd.memset(junk[:], 0.0)
            ps_warm = ps_pool.tile([128, 2 * QB], F32, tag="ps")
            for _ in range(6):
                nc.tensor.matmul(
                    ps_warm[:, 0:QB], lhsT=junk[:, 0:128], rhs=junk[:],
                    start=True, stop=True,
                )
            ones_col = persist.tile([128, 1], BF16, tag="ones")
            nc.gpsimd.memset(ones_col[:], 1.0)
            ident = persist.tile([128, 128], BF16, tag="ident")
            make_identity(nc, ident[:])
            bias_sb = persist.tile([D, 1], F32, tag="bias")
            nc.gpsimd.dma_start(bias_sb[:], bias_d[:])

            # peer slot register (host supplies 1 on even cores, 0 on odd)
            peer_reg = nc.sync.alloc_register("peer_slot")
            nc.sync.reg_load(peer_reg, peer_d[0:1, 0:1])
            peer_val = nc.sync.snap(peer_reg, donate=True, min_val=0, max_val=1)

            # ---- weight / x loads; w interleaved by first use ----
            w_sb = w_pool.tile([128, 3 * NE * 128], BF16, tag="w")
            WG = NE * 128  # one cg = 2048 cols

            def w_ap(cg, e):
                return w_sb[:, cg * WG + e * 128 : cg * WG + (e + 1) * 128]

            def w_load(cg, half, eng):
                lo = cg * WG + half * WG // 2
                eng.dma_start(w_sb[:, lo : lo + WG // 2], w_d[:, lo : lo + WG // 2])

            # k weights on the two fast queues; v/q weights head the scalar
            # queue (idle early); scalar x tiles only from e4 on (its first
            # transfers land late behind the engine preamble)
            w_load(0, 0, nc.sync)
            w_load(0, 1, nc.gpsimd)
            w_load(2, 0, nc.scalar)
            w_load(2, 1, nc.scalar)
            w_load(1, 0, nc.scalar)
            w_load(1, 1, nc.scalar)
            xt = {}
            x_engs = (nc.sync, nc.gpsimd, nc.sync, nc.gpsimd, nc.scalar,
                      nc.sync, nc.gpsimd, nc.scalar, nc.sync, nc.gpsimd,
                      nc.scalar, nc.sync, nc.gpsimd, nc.scalar, nc.sync,
                      nc.gpsimd)
            for e in range(NE):
                t = xt_pool.tile([128, HALF], BF16, tag="xt")
                x_engs[e].dma_start(t[:], x_d[e * 128 : (e + 1) * 128, :])
                xt[e] = t

            # ---- persistent activations ----
            qT = persist.tile([D, HALF], BF16, tag="qT")
            k_sb = persist.tile([D, S], BF16, tag="k_sb")  # [own kT | peer kT]
            vT_own = persist.tile([D, HALF], BF16, tag="vT_own")
            v_sb = persist.tile([128, (S // 128) * D], BF16, tag="v_sb")
            sums_stage = persist.tile([128, QB], F32, tag="sums_stage")
            o_stage = persist.tile([D, HALF], F32, tag="o_stage")

            def k_ap(c):  # kT chunk c (d on partitions); own 0-15, peer 16-31
                return k_sb[:, c * 128 : (c + 1) * 128]

            # ---- load phase: k-FULL + v-FULL chase the arriving x tiles
            # (both gate the combined K|V AllGather; v lags 2 e-chunks so
            # the wv DMA arrives) ----
            ps_ka = ps_pool.tile([128, 2 * QB], F32, tag="ps")
            ps_kb = ps_pool.tile([128, 2 * QB], F32, tag="ps")
            ps_va = ps_pool.tile([128, 2 * QB], F32, tag="ps")
            ps_vb = ps_pool.tile([128, 2 * QB], F32, tag="ps")
            pk = [ps_ka, ps_kb]
            pv = [ps_va, ps_vb]
            LAG = 2
            for step in range(NE + LAG):
                for cg, pd, e in ((0, pk, step), (2, pv, step - LAG)):
                    if 0 <= e < NE:
                        for blk in range(4):
                            nc.tensor.matmul(
                                pd[blk // 2][:, (blk % 2) * QB : (blk % 2 + 1) * QB],
                                lhsT=w_ap(cg, e),
                                rhs=xt[e][:, blk * QB : (blk + 1) * QB],
                                start=(e == 0),
                                stop=(e == NE - 1),
                            )
                if step < LAG:  # keep PE busy/warm while DMA ramps
                    for _ in range(4):
                        nc.tensor.matmul(
                            ps_warm[:, 0:QB], lhsT=junk[:, 0:128], rhs=junk[:],
                            start=True, stop=True,
                        )
            for i in range(2):
                nc.vector.tensor_copy(k_sb[:, i * 2 * QB : (i + 1) * 2 * QB], pk[i][:])
            for i in range(2):
                nc.vector.tensor_copy(
                    vT_own[:, i * 2 * QB : (i + 1) * 2 * QB], pv[i][:]
                )

            # ---- combined K|V exchange (1MB AllGather, one doorbell) ----
            cc_in = dram_pool.tile([D, S], BF16, tag="cc_in")
            cc_out = dram_pool.tile([2, D, S], BF16, tag="cc_out")
            nc.sync.dma_start(cc_in[:, HALF:S], vT_own[:])
            nc.gpsimd.dma_start(cc_in[:, 0:HALF], k_sb[:, 0:HALF])
            nc.gpsimd.collective_compute(
                "AllGather",
                mybir.AluOpType.bypass,
                replica_groups=GROUPS,
                ins=[cc_in.opt()],
                outs=[cc_out.opt()],
            )
            nc.sync.dma_start(k_sb[:, HALF:S], cc_out[peer_val][:, 0:HALF])
            for c in range(16):
                nc.sync.dma_start_transpose(
                    v_sb[:, (16 + c) * D : (17 + c) * D],
                    cc_out[peer_val][:, HALF + c * 128 : HALF + (c + 1) * 128],
                )

            # ---- attention machinery ----
            exp_regions = {}

            def scores_exp(qb, p):
                """Scores + exp for all 8 k-pairs of pass p, query block qb."""
                ex = exp_pool.tile([128, 16 * QB], BF16, tag="exp")
                exp_regions[(qb, p)] = ex
                q_ap = qT[:, qb * QB : (qb + 1) * QB]
                for kp in range(8):
                    ps = ps_pool.tile([128, 2 * QB], F32, tag="ps")
                    for half in range(2):
                        nc.tensor.matmul(
                            ps[:, half * QB : (half + 1) * QB],
                            lhsT=k_ap(16 * p + 2 * kp + half),
                            rhs=q_ap,
                            start=True,
                            stop=True,
                        )
                    nc.scalar.activation(
                        ex[:, kp * 2 * QB : (kp + 1) * 2 * QB], ps[:], AF.Exp,
                        scale=SCALE,
                    )

            def project_half(half, evac):
                """One 1024-row half of the q projection."""
                ps = ps_pool.tile([128, 2 * QB], F32, tag="ps")
                for e in range(NE):
                    for blk in range(2):
                        nc.tensor.matmul(
                            ps[:, blk * QB : (blk + 1) * QB],
                            lhsT=w_ap(1, e),
                            rhs=xt[e][:, (2 * half + blk) * QB : (2 * half + blk + 1) * QB],
                            start=(e == 0),
                            stop=(e == NE - 1),
                        )
                evac(ps[:])

            def pv_tree(qb, p):
                """PV accumulation + denominator tree for pass p, block qb."""
                ex = exp_regions.pop((qb, p))
                ps_o = ps_pool.tile([128, 2 * QB], F32, tag="ps")
                for kp in range(8):
                    for half in range(2):
                        c = 16 * p + 2 * kp + half
                        off = kp * 2 * QB + half * QB
                        nc.tensor.matmul(
                            ps_o[:, 0:QB],
                            lhsT=v_sb[:, c * D : (c + 1) * D],
                            rhs=ex[:, off : off + QB],
                            start=(kp == 0 and half == 0),
                            stop=(kp == 7 and half == 1),
                        )
                # output evacuation first: it is the pass-B critical tail
                o_sl = o_stage[:, qb * QB : (qb + 1) * QB]
                if p == 0:
                    nc.vector.tensor_copy(o_sl, ps_o[:, 0:QB])
                else:
                    nc.vector.tensor_add(o_sl, o_sl, ps_o[:, 0:QB])
                    nc.sync.dma_start(out_d[:, qb * QB : (qb + 1) * QB], o_sl)
                # halving tree; LA depends on kp0-3's exp, LB on kp4-7's, so
                # only M/N/C3 (~2.2us DVE) trail the final exp of the pass.
                W4, W2, W1 = 4 * QB, 2 * QB, QB
                cb = comb_pool.tile([128, 15 * QB], BF16, tag="comb")
                la, lb, m, n, c3 = (
                    cb[:, 0:W4],
                    cb[:, W4 : 2 * W4],
                    cb[:, 2 * W4 : 3 * W4],
                    cb[:, 3 * W4 : 3 * W4 + W2],
                    cb[:, 3 * W4 + W2 : 3 * W4 + W2 + W1],
                )
                nc.vector.tensor_add(la, ex[:, 0:W4], ex[:, W4 : 2 * W4])
                nc.vector.tensor_add(lb, ex[:, 2 * W4 : 3 * W4], ex[:, 3 * W4 : 4 * W4])
                nc.vector.tensor_add(m, la, lb)
                nc.vector.tensor_add(n, m[:, 0:W2], m[:, W2 : 2 * W2])
                nc.vector.tensor_add(c3, n[:, 0:W1], n[:, W1 : 2 * W1])
                ps_s = ps_pool.tile([128, 2 * QB], F32, tag="ps")
                nc.tensor.matmul(
                    ps_s[32 * qb : 32 * qb + 1, 0:QB],
                    lhsT=ones_col[:],
                    rhs=c3,
                    start=True,
                    stop=True,
                    tile_position=(0, 32 * qb),
                )
                s_sl = sums_stage[32 * qb : 32 * qb + 1, :]
                if p == 0:
                    nc.vector.tensor_copy(s_sl, ps_s[32 * qb : 32 * qb + 1, 0:QB])
                else:
                    nc.vector.tensor_add(s_sl, s_sl, ps_s[32 * qb : 32 * qb + 1, 0:QB])
                    nc.sync.dma_start(sums_d[0:1, qb * QB : (qb + 1) * QB], s_sl)

            # ---- pass A emission: q halves, exp stream chasing each ----
            project_half(
                0,
                lambda ps: nc.vector.tensor_scalar_add(
                    qT[:, 0 : 2 * QB], ps, bias_sb[:]
                ),
            )
            scores_exp(0, 0)
            scores_exp(1, 0)
            for c in range(16):  # own v chunks via PE transpose
                # bf16 [128, 2048] = same 4KB/partition as the fp32 tiles
                ps_t = ps_pool.tile([128, 4 * QB], BF16, tag="ps")
                nc.tensor.transpose(
                    ps_t[:, 0:128], vT_own[:, c * 128 : (c + 1) * 128], ident[:]
                )
                nc.vector.tensor_copy(v_sb[:, c * D : (c + 1) * D], ps_t[:, 0:128])
            project_half(
                1,
                lambda ps: nc.vector.tensor_scalar_add(
                    qT[:, 2 * QB : 4 * QB], ps, bias_sb[:]
                ),
            )
            scores_exp(2, 0)
            scores_exp(3, 0)

            # ---- interleave pass-A PV with pass-B scores/exp ----
            for qb in range(NQB):
                pv_tree(qb, 0)
                scores_exp(qb, 1)
            for qb in range(NQB):
                pv_tree(qb, 1)

    nc.compile()
    return nc


def _prep_inputs(x, W, b):
    """Host-side sharding prep: cast bf16, transpose to xT, pack w cg-major."""
    b_f = np.asarray(b, dtype=np.float32)
    bias_q = np.ascontiguousarray(b_f[0:D].reshape(D, 1))  # q bias column
    # W [E, 3D] -> [128p, cg(3), e(16), 128] with cg order (k, q, v)
    w4 = np.asarray(W).astype(ml_dtypes.bfloat16).reshape(NE, 128, 3, D)
    w_bf = np.ascontiguousarray(
        w4.transpose(1, 2, 0, 3)[:, [1, 0, 2], :, :].reshape(128, 3 * NE * D)
    )
    in_maps = []
    for bb in range(B):
        xt_full = np.ascontiguousarray(
            np.asarray(x[bb]).astype(ml_dtypes.bfloat16).T
        )  # [E, S]
        for h in range(2):
            xc = np.ascontiguousarray(xt_full[:, h * HALF : (h + 1) * HALF])
            peer = np.array([[1 - h]], dtype=np.uint32)
            in_maps.append(
                {"xt": xc, "w": w_bf, "bias_q": bias_q, "peer": peer}
            )
    return in_maps


def _run(in_maps, trace=False, trace_kwargs=None):
    if "nc" not in _CACHE:
        _CACHE["nc"] = _build()
    return run_bass_kernel_spmd(
        _CACHE["nc"],
        in_maps,
        list(range(N_CORES)),
        trace=trace,
        **(trace_kwargs or {}),
    )


def kernel(x, W, b):
    in_maps = _prep_inputs(x, W, b)
    res = None
    for attempt in range(3):
        try:
            res = _run(in_maps)
            break
        except Exception:
            if attempt == 2:
                raise
    bv = np.asarray(b, dtype=np.float32)[2 * D : 3 * D]  # v bias, host-applied
    out = np.empty((B, S, D), dtype=np.float32)
    for c in range(N_CORES):
        bb, h = c // 2, c % 2
        o_t = res.results[c]["out_t"]  # [D, HALF]
        sums = res.results[c]["sums"]  # [1, HALF]
        out[bb, h * HALF : (h + 1) * HALF, :] = (o_t / sums).T + bv
    return out
